# revision 16
# baseline (speedup 1.0000x reference)
"""Causal self-attention (B=2, S=2048, D=2048, H=16, HD=128) on 8 TRN2 cores.

Sharding: core c -> batch b = c//4, heads 4*(c%4)..4*(c%4)+3 (tensor-parallel
over heads within a batch; data-parallel over batch across core groups).

All PE inputs are bf16 (1 cycle/row at every tile size, vs fp32r's 4x
penalty below 256 output columns) with fp32 PSUM accumulation.

Q^T/K^T/V live in SBUF across both phases - no DRAM bounce.  Input DMA is
spread across the sync/scalar/vector/gpsimd/tensor queues so the prologue
weights arrive in parallel with the first x chunks, and the first s-block's
four Q accumulation groups are emitted interleaved per 128-row contraction
chunk so the PE starts as soon as the first chunks land.

Phase 2 runs q-blocks outer, heads inner; each q-block's output projection
(which needs all 4 heads) is emitted at (q-chunk, d-block) granularity into
later attention streams, with softmax-finalize jobs taking priority so the
last q-block's finalize latency is hidden instead of serializing the tail.
Host sums the 4 partials per batch.
"""

import math
from collections import deque

import ml_dtypes
import numpy as np

import concourse.bacc as bacc
import concourse.mybir as mybir
from concourse.tile import TileContext
from concourse.bass_utils import run_bass_kernel_spmd

B, S, D = 2, 2048, 2048
H, HD = 16, 128
ROPE_THETA = 10000.0

N_CORES = 8
CORES_PER_BATCH = 4
HPC = H // (N_CORES // B)  # heads per core = 4
HL = HPC * HD              # 512 local head-dim columns
NDC = D // 128             # 16 contraction chunks
NSB = S // 512             # 4 s-blocks
NKC = S // 128             # 16 k-chunks

F32 = mybir.dt.float32
BF16 = mybir.dt.bfloat16
AF = mybir.ActivationFunctionType
NPBF16 = ml_dtypes.bfloat16


def _mm(nc, out, lhsT, rhs, start, stop):
    nc.tensor.matmul(out, lhsT, rhs, start=start, stop=stop)


def _build():
    nc = bacc.Bacc("TRN2", target_bir_lowering=False, debug=False)

    xT = nc.dram_tensor("xT", [D, S], BF16, kind="ExternalInput")
    wq = nc.dram_tensor("wq", [D, HL], BF16, kind="ExternalInput")
    wk = nc.dram_tensor("wk", [D, HL], BF16, kind="ExternalInput")
    wv = nc.dram_tensor("wv", [D, HL], BF16, kind="ExternalInput")
    # wo pre-rearranged on host: wor[p, h*D + c] = Wo_local[h*128 + p, c]
    wor = nc.dram_tensor("wor", [128, HPC * D], BF16, kind="ExternalInput")
    cosT = nc.dram_tensor("cosT", [HD, S], F32, kind="ExternalInput")
    sinT = nc.dram_tensor("sinT", [HD, S], F32, kind="ExternalInput")
    pmatT = nc.dram_tensor("pmatT", [HD, HD], BF16, kind="ExternalInput")
    maskT = nc.dram_tensor("maskT", [128, 512], BF16, kind="ExternalInput")
    onesd = nc.dram_tensor("onesd", [128, 128], BF16, kind="ExternalInput")
    out = nc.dram_tensor("out", [S, D], F32, kind="ExternalOutput")

    with TileContext(nc) as tc:
        with (
            tc.tile_pool(name="pers", bufs=1) as pers,
            tc.tile_pool(name="consts", bufs=1) as consts,
            tc.tile_pool(name="psA", bufs=3, space="PSUM") as psA,
            tc.tile_pool(name="psB", bufs=3, space="PSUM") as psB,
            tc.tile_pool(name="psC", bufs=2, space="PSUM") as psC,
        ):
            qTs = [pers.tile([HD, S], BF16, name=f"qT{h}") for h in range(HPC)]
            kTs = [pers.tile([HD, S], BF16, name=f"kT{h}") for h in range(HPC)]
            # v_sb[:, kc, :] holds V rows kc*128..(kc+1)*128 for the 4 heads
            v_sb = pers.tile([128, NKC, HL], BF16, name="v_sb")
            wo_sb = pers.tile([128, HPC * D], BF16, name="wo_sb")
            ctxs = [pers.tile([HD, S], BF16, name=f"ctxT{h}") for h in range(HPC)]

            pmat_sb = consts.tile([HD, HD], BF16, name="pmat_sb")
            mask_sb = consts.tile([128, 512], BF16, name="mask_sb")
            ones_sb = consts.tile([128, 128], BF16, name="ones_sb")
            ones_col = ones_sb[:, 0:1]
            gpwarm = consts.tile([128, 128], F32, name="gpwarm")

            # ---------------- phase 1: projections + RoPE ----------------
            with (
                tc.tile_pool(name="wpool", bufs=1) as wpool,
                tc.tile_pool(name="xtp", bufs=24) as xtp,
                tc.tile_pool(name="st1", bufs=2) as st1,
            ):
                cos_sb = wpool.tile([HD, S], F32, name="cos_sb")
                sin_sb = wpool.tile([HD, S], F32, name="sin_sb")

                def load_w_chunk(w_d, dc, tag, queue):
                    wt = wpool.tile([128, HL], BF16, tag=f"{tag}{dc}", name="wt")
                    queue.dma_start(out=wt[:], in_=w_d[dc * 128:(dc + 1) * 128, :])
                    return wt

                def load_xt_chunk(dc, sb):
                    xt = xtp.tile([128, 512], BF16, tag="xt", name="xt")
                    nc.sync.dma_start(
                        out=xt[:], in_=xT[dc * 128:(dc + 1) * 128,
                                          sb * 512:(sb + 1) * 512])
                    return xt

                # prologue: x on the sync queue, wq/wv on the scalar queue,
                # consts + wk + RoPE tables on the gpsimd queue, so the Q
                # groups' operands stream in parallel at ~2 chunks/us.
                nc.gpsimd.dma_start(out=pmat_sb[:], in_=pmatT[:])
                nc.gpsimd.dma_start(out=mask_sb[:], in_=maskT[:])
                nc.gpsimd.dma_start(out=ones_sb[:], in_=onesd[:])
                xts = []
                wq_t, wk_t, wv_t = [], [], []
                for dc in range(NDC):
                    xts.append(load_xt_chunk(dc, 0))
                    wq_t.append(load_w_chunk(wq, dc, "wq", nc.scalar))
                    wk_t.append(load_w_chunk(wk, dc, "wk", nc.gpsimd))
                    wv_t.append(load_w_chunk(
                        wv, dc, "wv", nc.scalar if dc % 2 else nc.gpsimd))
                nc.sync.dma_start(out=cos_sb[:], in_=cosT[:])
                nc.sync.dma_start(out=sin_sb[:], in_=sinT[:])
                # warm up the GpSimd library load off the critical path
                nc.gpsimd.partition_broadcast(gpwarm[:], cos_sb[0:1, 0:128])
                nc.scalar.dma_start(out=wo_sb[:], in_=wor[:])

                finishers = deque()

                def emit_finisher():
                    kind, args = finishers.popleft()
                    if kind == "qk":
                        ps, qraw, dst, sl = args
                        rot = psB.tile([128, 512], F32, tag="b", name="rot")
                        _mm(nc, rot[:], pmat_sb[:], qraw[:], start=True, stop=True)
                        acos = st1.tile([128, 512], F32, tag="acos", name="acos")
                        nc.vector.tensor_mul(acos[:], qraw[:], cos_sb[:, sl])
                        rsin = st1.tile([128, 512], F32, tag="rsin", name="rsin")
                        nc.vector.tensor_mul(rsin[:], rot[:], sin_sb[:, sl])
                        nc.vector.tensor_add(dst[:, sl], rsin[:], acos[:])
                    else:
                        ps, sb, sc = args
                        nc.scalar.activation(v_sb[:, 4 * sb + sc, :], ps[:], AF.Copy)

                def finish_group(ps, dst, sl):
                    qraw = st1.tile([128, 512], BF16, tag="qraw", name="qraw")
                    nc.scalar.activation(qraw[:], ps[:], AF.Copy)
                    finishers.append(("qk", (ps, qraw, dst, sl)))
                    if len(finishers) > 1:
                        emit_finisher()

                # sb=0 Q groups: 4 accumulators interleaved per dc chunk
                # (3 psA bufs + 1 borrowed from psC, which is idle until
                # phase 2) so the PE tracks the incoming DMA stream.
                qps = [psA.tile([128, 512], F32, tag="a", name="qps")
                       for _ in range(HPC - 1)]
                qps.append(psC.tile([128, 512], F32, tag="c", name="qps3"))
                for dc in range(NDC):
                    for h in range(HPC):
                        _mm(nc, qps[h][:],
                            wq_t[dc][:, h * HD:(h + 1) * HD],
                            xts[dc][:],
                            start=(dc == 0), stop=(dc == NDC - 1))
                sl0 = slice(0, 512)
                for h in range(HPC):
                    finish_group(qps[h], qTs[h], sl0)

                for sb in range(NSB):
                    sl = slice(sb * 512, (sb + 1) * 512)
                    if sb > 0:
                        xts = [load_xt_chunk(dc, sb) for dc in range(NDC)]

                    groups = ((wk_t, kTs),) if sb == 0 else ((wq_t, qTs), (wk_t, kTs))
                    for w_t, dst in groups:
                        for h in range(HPC):
                            ps = psA.tile([128, 512], F32, tag="a", name="ps")
                            for dc in range(NDC):
                                _mm(nc, ps[:],
                                    w_t[dc][:, h * HD:(h + 1) * HD],
                                    xts[dc][:],
                                    start=(dc == 0), stop=(dc == NDC - 1))
                            finish_group(ps, dst[h], sl)

                    for sc in range(4):
                        ps = psA.tile([128, 512], F32, tag="a", name="ps")
                        for dc in range(NDC):
                            _mm(nc, ps[:],
                                xts[dc][:, sc * 128:(sc + 1) * 128],
                                wv_t[dc][:],
                                start=(dc == 0), stop=(dc == NDC - 1))
                        finishers.append(("v", (ps, sb, sc)))
                        if len(finishers) > 1:
                            emit_finisher()
                while finishers:
                    emit_finisher()

            # ---------- phase 2+3: attention + output projection ----------
            with (
                tc.tile_pool(name="pp", bufs=8) as pp,
                tc.tile_pool(name="sm", bufs=3) as sm,
                tc.tile_pool(name="outp", bufs=3) as outp,
            ):
                lagq = deque()
                fin = deque()    # (h, qb, lps, pv)
                lpv_done = set()  # ids of lps tiles whose accumulation is fully emitted

                def emit_lpv(job):
                    lps, pv, pt, vtc, ncols, first, last = job
                    _mm(nc, lps[:, 512 - ncols:], ones_col, pt[:, :ncols],
                        start=first, stop=last)
                    _mm(nc, pv[:, 512 - ncols:], vtc, pt[:, :ncols],
                        start=first, stop=last)
                    if last:
                        lpv_done.add(id(lps))

                def emit_finalize(job):
                    # broadcast raw l via GpSimd, then approx reciprocal +
                    # scale on DVE, fully off the PE path
                    h, qb, lps, pv = job
                    lsb = sm.tile([1, 512], F32, tag="lsb", name="lsb")
                    # clamp away from 0 so a fully-flushed fp8 row yields
                    # ctx=0 instead of inf/NaN
                    nc.vector.tensor_scalar_max(lsb[:], lps[:], 1e-20)
                    repsb = sm.tile([128, 512], F32, tag="repsb", name="repsb")
                    nc.gpsimd.partition_broadcast(repsb[:], lsb[:])
                    rcps = sm.tile([128, 512], F32, tag="rcps", name="rcps")
                    rcp = sm.tile([128, 512], F32, tag="rcp", name="rcp")
                    nc.vector.reciprocal_approx_accurate(rcp[:], repsb[:], rcps[:])
                    nc.vector.tensor_mul(ctxs[h][:, qb * 512:(qb + 1) * 512],
                                         pv[:], rcp[:])

                def emit_outproj(job):
                    qc, db = job
                    ops = psA.tile([128, 512], F32, tag="a", name="ops")
                    for h in range(HPC):
                        _mm(nc, ops[:],
                            ctxs[h][:, qc * 128:(qc + 1) * 128],
                            wo_sb[:, h * D + db * 512: h * D + (db + 1) * 512],
                            start=(h == 0), stop=(h == HPC - 1))
                    osb = outp.tile([128, 512], F32, tag="osb", name="osb")
                    if (qc + db) % 2:
                        nc.scalar.activation(osb[:], ops[:], AF.Copy)
                    else:
                        nc.vector.tensor_copy(osb[:], ops[:])
                    nc.sync.dma_start(
                        out=out[qc * 128:(qc + 1) * 128,
                                db * 512:(db + 1) * 512],
                        in_=osb[:])

                outproj_ready = [0] * NSB  # finalizes emitted per q-block
                outproj_q = deque()        # (qc, db) jobs whose ctx is complete

                def pop_finalize():
                    if fin and id(fin[0][2]) in lpv_done:
                        job = fin.popleft()
                        emit_finalize(job)
                        qb = job[1]
                        outproj_ready[qb] += 1
                        if outproj_ready[qb] == HPC:
                            outproj_q.extend((qc, db)
                                             for qc in range(4 * qb, 4 * qb + 4)
                                             for db in range(D // 512))
                        return True
                    return False

                for qb in range(NSB):
                    for h in range(HPC):
                        nk = 4 * qb + 4
                        hs = slice(h * HD, (h + 1) * HD)
                        lps = psC.tile([1, 512], F32, tag="c", name="lps")
                        pv = psB.tile([128, 512], F32, tag="b", name="pv")
                        pop_finalize()
                        for kc in range(nk):
                            j = kc - 4 * qb
                            ncols = 512 if j < 0 else 512 - 128 * j
                            q0 = qb * 512 + 512 - ncols
                            sps = psA.tile([128, 512], F32, tag="a", name="sps")
                            _mm(nc, sps[:, :ncols],
                                kTs[h][:, kc * 128:(kc + 1) * 128],
                                qTs[h][:, q0:(qb + 1) * 512],
                                start=True, stop=True)
                            pt = pp.tile([128, 512], BF16, tag="pt", name="pt")
                            nc.scalar.activation(pt[:, :ncols], sps[:, :ncols], AF.Exp)
                            if j >= 0:
                                nc.vector.tensor_mul(pt[:, :ncols], pt[:, :ncols],
                                                     mask_sb[:, :ncols])
                            lagq.append((lps, pv, pt, v_sb[:, kc, hs], ncols,
                                         kc == 0, kc == nk - 1))
                            while len(lagq) > 2:
                                emit_lpv(lagq.popleft())
                            if kc % 2 == 1:
                                if not pop_finalize() and outproj_q:
                                    emit_outproj(outproj_q.popleft())
                        fin.append((h, qb, lps, pv))
                while lagq:
                    emit_lpv(lagq.popleft())
                while fin:
                    pop_finalize()
                    while outproj_q:
                        emit_outproj(outproj_q.popleft())

    nc.compile()
    return nc


_NC_CACHE = None


def _get_nc():
    global _NC_CACHE
    if _NC_CACHE is None:
        _NC_CACHE = _build()
    return _NC_CACHE


def _host_tables():
    # Replicate reference RoPE tables in float32 arithmetic, transposed.
    inv_freq = np.float32(1.0) / np.power(
        np.float32(ROPE_THETA), np.arange(0, HD, 2).astype(np.float32) / np.float32(HD)
    )
    pos = np.arange(S, dtype=np.float32)
    freqs = pos[:, None] * inv_freq[None, :]
    angles = np.concatenate([freqs, freqs], axis=1)  # [S, HD]
    cos_t = np.ascontiguousarray(np.cos(angles).astype(np.float32).T)  # [HD, S]
    sin_t = np.ascontiguousarray(np.sin(angles).astype(np.float32).T)
    # rotate_half as a left-multiply matrix P: (P q)[2i] = -q[2i+1], [2i+1] = q[2i].
    # matmul computes lhsT.T @ rhs, so feed P.T.
    pmat = np.zeros((HD, HD), dtype=np.float32)
    for i in range(HD // 2):
        pmat[2 * i, 2 * i + 1] = -1.0
        pmat[2 * i + 1, 2 * i] = 1.0
    pmat_t = np.ascontiguousarray(pmat.T).astype(NPBF16)
    mask = (np.arange(128)[:, None] <= np.arange(512)[None, :]).astype(NPBF16)
    return cos_t, sin_t, pmat_t, mask


_ONES = np.ones((128, 128), dtype=NPBF16)


def kernel(x, Wq, Wk, Wv, Wo):
    x = np.asarray(x, dtype=np.float32)
    Wq = np.asarray(Wq, dtype=np.float32)
    Wk = np.asarray(Wk, dtype=np.float32)
    Wv = np.asarray(Wv, dtype=np.float32)
    Wo = np.asarray(Wo, dtype=np.float32)

    results = _run_device(x, Wq, Wk, Wv, Wo)

    out = np.empty((B, S, D), dtype=np.float32)
    for b in range(B):
        acc = results[b * CORES_PER_BATCH]["out"]
        for i in range(1, CORES_PER_BATCH):
            acc = acc + results[b * CORES_PER_BATCH + i]["out"]
        out[b] = acc
    return out


def _make_in_maps(x, Wq, Wk, Wv, Wo):
    cos_t, sin_t, pmat_t, mask = _host_tables()
    scale = np.float32(1.0 / math.sqrt(HD))
    wq_scaled = (Wq * scale).astype(np.float32)
    xTb = [np.ascontiguousarray(x[b].T).astype(NPBF16) for b in range(B)]
    in_maps = []
    for c in range(N_CORES):
        b = c // CORES_PER_BATCH
        g = c % CORES_PER_BATCH
        hs = slice(g * HL, (g + 1) * HL)
        wo_local = Wo[hs, :]  # [HL, D]
        # wor[p, h*D + c] = wo_local[h*128 + p, c]
        wor = np.ascontiguousarray(
            wo_local.reshape(HPC, 128, D).transpose(1, 0, 2).reshape(128, HPC * D)
        ).astype(NPBF16)
        in_maps.append({
            "xT": xTb[b],
            "wq": np.ascontiguousarray(wq_scaled[:, hs]).astype(NPBF16),
            "wk": np.ascontiguousarray(Wk[:, hs]).astype(NPBF16),
            "wv": np.ascontiguousarray(Wv[:, hs]).astype(NPBF16),
            "wor": wor,
            "cosT": cos_t,
            "sinT": sin_t,
            "pmatT": pmat_t,
            "maskT": mask,
            "onesd": _ONES,
        })
    return in_maps


def _run_device(x, Wq, Wk, Wv, Wo, trace=False):
    nc = _get_nc()
    in_maps = _make_in_maps(x, Wq, Wk, Wv, Wo)
    res = run_bass_kernel_spmd(nc, in_maps, core_ids=list(range(N_CORES)), trace=trace)
    if trace:
        return res
    return res.results


def run_traced(x, Wq, Wk, Wv, Wo):
    """Run with NTFF tracing; returns (full_output, BassKernelResults)."""
    res = _run_device(np.asarray(x, np.float32), np.asarray(Wq, np.float32),
                      np.asarray(Wk, np.float32), np.asarray(Wv, np.float32),
                      np.asarray(Wo, np.float32), trace=True)
    out = np.empty((B, S, D), dtype=np.float32)
    for b in range(B):
        acc = res.results[b * CORES_PER_BATCH]["out"]
        for i in range(1, CORES_PER_BATCH):
            acc = acc + res.results[b * CORES_PER_BATCH + i]["out"]
        out[b] = acc
    return out, res
